# revision 21
# baseline (speedup 1.0000x reference)
"""Trainium2 Bass kernel for nn_Attention_84473416778449.

Reference computation (B=2, S=2048, D=1024, H=16, HD=64, fp32):
    q/k/v = x @ w{q,k,v}.T ; RoPE(q, k) ; causal softmax attention ; out @ wo.T

Sharding: 8 cores = (batch 2) x (head-group 4). Each core computes 4 heads of
one batch end-to-end and a partial output projection over its 256 channels;
the host sums the 4 partials per batch.

v5 structure (evidence-driven through four traced iterations):
  - fp16 host-packed inputs; x blocks load as single dma descriptors.
  - Loads spread over the three DMA-capable queues (sync/scalar/gpsimd),
    ordered by first-need so the PE never bubbles after the ramp
    (per-queue dma bandwidth measured ~115GB/s).
  - PE warm-up matmuls bridge the framework preamble so the HAM clock gate
    reaches 8/8 before real work.
  - Score matmuls for a head pair go to disjoint PE row groups
    (tile_position (0,0)/(64,0)) and run concurrently (measured 4ns apart);
    both write halves of one 2-bank psum tile so ONE batched exp covers the
    pair (halves the ACT per-instruction overhead).
  - V tiles carry 64 replicated ones-columns per head, so the PV matmul
    emits the softmax denominator replicated across psum rows 64:128 and
    normalization is just reciprocal+multiply on the DVE (no psum row copy,
    no gpsimd partition broadcast on the critical path).
  - The RoPE matmul reuses its own chain's psA tile (dead after the cos/sin
    multiplies) and is emitted a few chunks late so it never head-of-line
    blocks the PE queue.
"""
import sys

if "/opt/trn_rl_repo" not in sys.path:
    sys.path.insert(0, "/opt/trn_rl_repo")

import numpy as np

import concourse.bass as bass
import concourse.mybir as mybir
import concourse.tile as tile
from concourse import bacc
from concourse.bass_utils import run_bass_kernel_spmd

B, S, D, H, HD = 2, 2048, 1024, 16, 64
NCORES = 8
GROUPS = 4            # head groups
GH = H // GROUPS      # heads per group = 4
GC = GH * HD          # channels per group = 256
KT = D // 128         # 8 k-tiles over D
ST = S // 128         # 16 s-tiles
QB = 4                # sq blocks of 512
QW = S // QB          # 512
XW = KT * QW          # 4096: packed x block width

f32 = mybir.dt.float32
MMDT = mybir.dt.float16   # matmul-operand dtype
Exp = mybir.ActivationFunctionType.Exp
Copy = mybir.ActivationFunctionType.Copy

_cache = {}


def _build():
    nc = bacc.Bacc("TRN2", num_devices=NCORES)

    # host-packed: row-block cb holds [p, kt*QW + j] = x[b].T[kt*128+p, cb*QW+j]
    xB = nc.dram_tensor("xB", [QB * 128, XW], MMDT, kind="ExternalInput").ap()
    # host-packed: [p, kt*GC + c] = w.T[kt*128 + p, c]
    wqs = nc.dram_tensor("wqs", [128, KT * GC], MMDT, kind="ExternalInput").ap()
    wks = nc.dram_tensor("wks", [128, KT * GC], MMDT, kind="ExternalInput").ap()
    wvs = nc.dram_tensor("wvs", [128, KT * GC], MMDT, kind="ExternalInput").ap()
    woT = nc.dram_tensor("woT", [GC, D], MMDT, kind="ExternalInput").ap()
    cs2 = nc.dram_tensor("cs2", [128, S], MMDT, kind="ExternalInput").ap()
    sn2 = nc.dram_tensor("sn2", [128, S], MMDT, kind="ExternalInput").ap()
    out = nc.dram_tensor("out", [S, D], MMDT, kind="ExternalOutput").ap()

    with tile.TileContext(nc) as tc:
        with tc.tile_pool(name="persist", bufs=1) as pp, \
             tc.tile_pool(name="rope", bufs=3) as rp, \
             tc.tile_pool(name="probs", bufs=6) as wp, \
             tc.tile_pool(name="outsb", bufs=2) as op_, \
             tc.tile_pool(name="small", bufs=3) as sp:

            xb = [pp.tile([128, XW], MMDT, tag=f"xb{cb}", name=f"xb{cb}")
                  for cb in range(QB)]

            def xs(kt, cb):
                return xb[cb][:, kt * QW:(kt + 1) * QW]

            def load_w(src, eng):
                t = pp.tile([128, KT * GC], MMDT, tag=f"w{src.tensor.name}",
                            name=f"w{src.tensor.name}")
                eng.dma_start(t[:], src)
                return t

            # ---- loads spread across the three DMA queues by first-need ----
            warm = pp.tile([128, 128], MMDT, tag="warm")
            nc.gpsimd.memset(warm[:], 0.0)
            warm2 = pp.tile([128, QW + 128], MMDT, tag="warm2")
            nc.gpsimd.memset(warm2[:], 0.0)
            nc.gpsimd.dma_start(xb[0][:, 0:XW // 2], xB[0:128, 0:XW // 2])
            wq_s = load_w(wqs, nc.sync)
            nc.scalar.dma_start(xb[0][:, XW // 2:XW], xB[0:128, XW // 2:XW])
            cs_sb = pp.tile([128, S], MMDT, tag="cs")
            nc.sync.dma_start(cs_sb[:], cs2[:])
            sn_sb = pp.tile([128, S], MMDT, tag="sn")
            nc.scalar.dma_start(sn_sb[:], sn2[:])
            wk_s = load_w(wks, nc.gpsimd)
            wv_s = load_w(wvs, nc.scalar)
            wo_s = []
            for kt in range(2):
                t = pp.tile([128, D], MMDT, tag=f"wo{kt}", name=f"wo{kt}")
                nc.sync.dma_start(t[:], woT[kt * 128:(kt + 1) * 128, :])
                wo_s.append(t)
            nc.gpsimd.dma_start(xb[1][:], xB[128:256, :])
            nc.gpsimd.dma_start(xb[2][:], xB[256:384, :])
            nc.gpsimd.dma_start(xb[3][:], xB[384:512, :])

            # ---- PE warm-up: dummy matmuls while the first DMAs land ------
            with tc.tile_pool(name="psW", bufs=1, space="PSUM") as psW:
                wps = psW.tile([128, QW], f32, tag="wps")
                for _ in range(20):
                    nc.tensor.matmul(wps[:, 0:128], warm[:], warm[:],
                                     start=True, stop=True)
                for _ in range(26):
                    nc.tensor.matmul(wps[:], warm2[:, QW:QW + 128],
                                     warm2[:, 0:QW],
                                     start=True, stop=True)

            # ---- constants -------------------------------------------------
            cscratch = pp.tile([128, 128], f32, tag="cscratch")
            nc.gpsimd.memset(cscratch[:], 0.0)
            for blk in range(2):
                sub = cscratch[blk * 64:(blk + 1) * 64,
                               blk * 64:(blk + 1) * 64]
                nc.gpsimd.affine_select(   # -1 where p - f == 32
                    out=sub, in_=sub, pattern=[[-1, 64]], base=-32,
                    channel_multiplier=1,
                    compare_op=mybir.AluOpType.not_equal, fill=-1.0)
                nc.gpsimd.affine_select(   # +1 where f - p == 32
                    out=sub, in_=sub, pattern=[[1, 64]], base=-32,
                    channel_multiplier=-1,
                    compare_op=mybir.AluOpType.not_equal, fill=1.0)
            rt2 = pp.tile([128, 128], MMDT, tag="rt2")
            nc.vector.tensor_copy(rt2[:], cscratch[:])

            qT = [[pp.tile([128, QW], MMDT, tag=f"qT{i}_{b}",
                           name=f"qT{i}_{b}") for b in range(QB)]
                  for i in range(2)]
            kTt = [[pp.tile([128, QW], MMDT, tag=f"kT{i}_{b}",
                            name=f"kT{i}_{b}") for b in range(QB)]
                   for i in range(2)]
            attnT = [[pp.tile([128, QW], MMDT, tag=f"aT{i}_{b}",
                              name=f"aT{i}_{b}") for b in range(QB)]
                     for i in range(2)]
            # per head h: cols h*128+0:64 = v payload, h*128+64:128 = ones
            # (PV then emits the denominator replicated on psum rows 64:128)
            v_sb = [pp.tile([128, 4 * 128], MMDT, tag=f"v{i}",
                            name=f"v{i}") for i in range(ST)]
            ones256 = pp.tile([128, 256], f32, tag="ones256")
            nc.gpsimd.memset(ones256[:], 1.0)
            for vt in v_sb:
                nc.vector.tensor_copy(
                    vt[:].rearrange("p (h e) -> p h e", e=128)[:, :, HD:128],
                    ones256[:].rearrange("p (h e) -> p h e", e=HD))

            with tc.tile_pool(name="psA", bufs=2, space="PSUM") as psA, \
                 tc.tile_pool(name="psS", bufs=2, space="PSUM") as psS, \
                 tc.tile_pool(name="psO", bufs=2, space="PSUM") as psO:

                def qk_chain_chunks(w_src, dst, hp, sb):
                    """[mm x8, fin_dve] plus a separate fin_rope chunk that the
                    caller emits a few chunks later (it holds the PE rope
                    matmul, which must not wait at the head of the PE queue).
                    The rope matmul reuses the chain's own psA tile."""
                    cols = slice(sb * QW, (sb + 1) * QW)
                    box = {}

                    def mk_mm(kt):
                        def f():
                            if kt == 0:
                                box['pq'] = psA.tile(
                                    [128, QW], f32, tag="proj",
                                    name=f"pq{hp}_{sb}")
                            base = kt * GC + hp * 128
                            if kt == 0:
                                nc.tensor.matmul(
                                    box['pq'][:],
                                    w_src[:, base:base + 128],
                                    xs(kt, sb),
                                    start=True, stop=False,
                                    skip_group_check=True)
                            else:
                                nc.tensor.matmul(
                                    box['pq'][0:64, :],
                                    w_src[:, base:base + 64],
                                    xs(kt, sb),
                                    start=False, stop=(kt == KT - 1),
                                    tile_position=(0, 0),
                                    skip_group_check=True)
                                nc.tensor.matmul(
                                    box['pq'][64:128, :],
                                    w_src[:, base + 64:base + 128],
                                    xs(kt, sb),
                                    start=False, stop=(kt == KT - 1),
                                    tile_position=(0, 64),
                                    skip_group_check=True)
                        return f

                    def fin_dve():
                        pq = box['pq']
                        box['tcs'] = rp.tile([128, QW], MMDT, tag="tcs",
                                             name=f"tcs{hp}_{sb}")
                        nc.vector.tensor_tensor(
                            out=box['tcs'][:], in0=pq[:], in1=cs_sb[:, cols],
                            op=mybir.AluOpType.mult)
                        box['tsn'] = rp.tile([128, QW], MMDT, tag="tsn",
                                             name=f"tsn{hp}_{sb}")
                        nc.vector.tensor_tensor(
                            out=box['tsn'][:], in0=pq[:], in1=sn_sb[:, cols],
                            op=mybir.AluOpType.mult)

                    def fin_rope():
                        pq = box['pq']
                        nc.tensor.matmul(pq[:], rt2[:], box['tsn'][:],
                                         start=True, stop=True)
                        nc.vector.tensor_tensor(
                            out=dst[hp][sb][:], in0=pq[:], in1=box['tcs'][:],
                            op=mybir.AluOpType.add)
                    return [mk_mm(kt) for kt in range(KT)] + [fin_dve], fin_rope

                def v_chain_chunks(st):
                    box = {}

                    def mk_mm(kt):
                        def f():
                            if kt == 0:
                                box['pv'] = psA.tile(
                                    [128, QW], f32, tag="proj",
                                    name=f"pv_{st}")
                            xsl = xs(kt, st // 4)[:, (st % 4) * 128:
                                                  (st % 4) * 128 + 128]
                            if kt == 0:
                                nc.tensor.matmul(
                                    box['pv'][:, 0:GC],
                                    xsl,
                                    wv_s[:, kt * GC:(kt + 1) * GC],
                                    start=True, stop=False,
                                    skip_group_check=True)
                            else:
                                nc.tensor.matmul(
                                    box['pv'][0:64, 0:GC],
                                    xsl[:, 0:64],
                                    wv_s[:, kt * GC:(kt + 1) * GC],
                                    start=False, stop=(kt == KT - 1),
                                    tile_position=(0, 0),
                                    skip_group_check=True)
                                nc.tensor.matmul(
                                    box['pv'][64:128, 0:GC],
                                    xsl[:, 64:128],
                                    wv_s[:, kt * GC:(kt + 1) * GC],
                                    start=False, stop=(kt == KT - 1),
                                    tile_position=(0, 64),
                                    skip_group_check=True)
                        return f

                    def fin():
                        vhe = v_sb[st][:].rearrange("p (h e) -> p h e", e=128)
                        nc.scalar.activation(
                            vhe[:, :, 0:HD],
                            box['pv'][:, 0:GC].rearrange(
                                "p (h d) -> p h d", d=HD),
                            Copy)
                    return [mk_mm(kt) for kt in range(KT)] + [fin]

                def proj_chunks(sbx):
                    """Chunk list for block sbx with each chain's rope matmul
                    deferred into the following chain's matmul stream."""
                    chunks = []
                    pending = []

                    def flush():
                        while pending:
                            chunks.append(pending.pop(0))

                    for hp in range(2):
                        for w_src, dst in ((wq_s, qT), (wk_s, kTt)):
                            main, fin_rope = qk_chain_chunks(
                                w_src, dst, hp, sbx)
                            chunks += main[:3]
                            flush()
                            chunks += main[3:]
                            pending.append(fin_rope)
                    for st in range(sbx * 4, sbx * 4 + 4):
                        vc = v_chain_chunks(st)
                        chunks += vc[:3]
                        flush()
                        chunks += vc[3:]
                    return chunks

                def out_chunks(st, on_dve=False):
                    box = {}

                    def mms(db):
                        def f():
                            box[db] = psA.tile([128, QW], f32, tag="proj",
                                               name=f"pc_{st}_{db}")
                            if db == 0:
                                box['ob'] = op_.tile([128, 2 * QW], MMDT,
                                                     tag="outsb",
                                                     name=f"ob_{st}")
                            for kt in range(2):
                                sl = attnT[kt][st // 4][:, (st % 4) * 128:
                                                        (st % 4) * 128 + 128]
                                if kt == 0:
                                    nc.tensor.matmul(
                                        box[db][:], sl,
                                        wo_s[kt][:, db * QW:(db + 1) * QW],
                                        start=True, stop=False,
                                        skip_group_check=True)
                                    continue
                                nc.tensor.matmul(
                                    box[db][0:64, :],
                                    sl[:, 0:64],
                                    wo_s[kt][:, db * QW:(db + 1) * QW],
                                    start=False, stop=(kt == 1),
                                    tile_position=(0, 0),
                                    skip_group_check=True)
                                nc.tensor.matmul(
                                    box[db][64:128, :],
                                    sl[:, 64:128],
                                    wo_s[kt][:, db * QW:(db + 1) * QW],
                                    start=False, stop=(kt == 1),
                                    tile_position=(0, 64),
                                    skip_group_check=True)
                        return f

                    def cp(db):
                        def f():
                            if on_dve:
                                nc.vector.tensor_copy(
                                    box['ob'][:, db * QW:(db + 1) * QW],
                                    box[db][:])
                            else:
                                nc.scalar.activation(
                                    box['ob'][:, db * QW:(db + 1) * QW],
                                    box[db][:], Copy)
                            if db == 1:
                                nc.sync.dma_start(
                                    out[st * 128:(st + 1) * 128, :],
                                    box['ob'][:])
                        return f
                    return [mms(0), cp(0), mms(1), cp(1)]

                def merge_prop(a, b):
                    res, ia, ib = [], 0, 0
                    la, lb = max(len(a), 1), max(len(b), 1)
                    while ia < len(a) or ib < len(b):
                        if ib >= len(b) or (ia < len(a) and
                                            ia * lb <= ib * la):
                            res.append(a[ia]); ia += 1
                        else:
                            res.append(b[ib]); ib += 1
                    return res

                def emit_attention(qb, fillers):
                    nsk = (qb + 1) * 4
                    steps_total = 2 * nsk
                    step = 0
                    fi = 0
                    for hp in range(2):
                        hA, hB = 2 * hp, 2 * hp + 1
                        poA = psO.tile([128, QW], f32, tag="pvacc",
                                       name=f"poA_{hp}_{qb}")
                        poB = psO.tile([128, QW], f32, tag="pvacc",
                                       name=f"poB_{hp}_{qb}")

                        def pv_pair(kt, c0, cw, prt):
                            nc.tensor.matmul(
                                poA[:, c0:QW],
                                v_sb[kt][:, hA * 128:hA * 128 + 128],
                                prt[:, 0:cw],
                                start=(kt == 0), stop=(kt == nsk - 1))
                            nc.tensor.matmul(
                                poB[:, c0:QW],
                                v_sb[kt][:, hB * 128:hB * 128 + 128],
                                prt[:, QW:QW + cw],
                                start=(kt == 0), stop=(kt == nsk - 1))

                        for kt in range(nsk):
                            c0 = max(0, kt * 128 - qb * QW)
                            cw = QW - c0
                            ps2 = psS.tile([128, 2 * QW], f32, tag="score",
                                           name=f"sc_{hp}_{qb}_{kt}")
                            nc.tensor.matmul(
                                ps2[:, 0:cw],
                                kTt[hp][kt // 4][0:64,
                                                 (kt % 4) * 128:
                                                 (kt % 4) * 128 + 128],
                                qT[hp][qb][0:64, c0:QW],
                                start=True, stop=True,
                                tile_position=(0, 0))
                            nc.tensor.matmul(
                                ps2[:, QW:QW + cw],
                                kTt[hp][kt // 4][64:128,
                                                 (kt % 4) * 128:
                                                 (kt % 4) * 128 + 128],
                                qT[hp][qb][64:128, c0:QW],
                                start=True, stop=True,
                                tile_position=(64, 0))
                            prt = wp.tile([128, 2 * QW], MMDT, tag="probs",
                                          name=f"pr_{hp}_{qb}_{kt}")
                            nc.scalar.activation(
                                prt[:].rearrange(
                                    "p (b j) -> p b j", b=2)[:, :, 0:cw],
                                ps2[:].rearrange(
                                    "p (b j) -> p b j", b=2)[:, :, 0:cw],
                                Exp, scale=0.125)
                            if kt >= nsk - 4:
                                for pv in (prt[:, 0:cw],
                                           prt[:, QW:QW + cw]):
                                    nc.gpsimd.affine_select(
                                        out=pv, in_=pv,
                                        pattern=[[1, cw]], base=0,
                                        channel_multiplier=-1,
                                        compare_op=mybir.AluOpType.is_ge,
                                        fill=0.0)
                            # one filler matmul covers the exp latency before
                            # this step's PV; the rest follow the PV
                            step += 1
                            want = (len(fillers) * step) // steps_total
                            if fi < want:
                                fillers[fi]()
                                fi += 1
                            pv_pair(kt, c0, cw, prt)
                            while fi < want:
                                fillers[fi]()
                                fi += 1
                        for h, po in ((hA, poA), (hB, poB)):
                            den_s = sp.tile([64, QW], f32, tag="dens")
                            nc.vector.tensor_copy(den_s[:], po[64:128, :])
                            rcb = sp.tile([64, QW], f32, tag="recb")
                            nc.vector.reciprocal_approx_fast(
                                out=rcb[:], in_=den_s[:])
                            nc.vector.tensor_tensor(
                                out=attnT[hp][qb][(h % 2) * 64:
                                                  (h % 2) * 64 + 64, :],
                                in0=po[0:HD, :], in1=rcb[:],
                                op=mybir.AluOpType.mult)
                    while fi < len(fillers):
                        fillers[fi]()
                        fi += 1

                # sb=0 projections are a dense block (nothing to hide behind)
                for ch in proj_chunks(0):
                    ch()
                for sb in range(QB):
                    steps = 2 * (sb + 1) * 4
                    outs = []
                    if sb > 0:
                        for st in range((sb - 1) * 4, sb * 4):
                            outs += out_chunks(st)
                    rest = []
                    projs = []
                    if sb + 1 < QB:
                        pch = proj_chunks(sb + 1)
                        # cap interleave rate so deferred DVE work cannot
                        # back up behind the PE racing through fillers
                        ncap = max(0, 2 * steps - len(outs))
                        projs = pch[:ncap]
                        rest = pch[ncap:]
                    emit_attention(sb, merge_prop(projs, outs))
                    for ch in rest:
                        ch()
                for st in range((QB - 1) * 4, QB * 4):
                    for ch in out_chunks(st, on_dve=True):
                        ch()

    nc.compile()
    return nc


def _shard_inputs(x, cos, sin, wq, wk, wv, wo):
    x16 = np.asarray(x, dtype=np.float16)
    cosT = np.asarray(cos, np.float32).reshape(S, HD).T
    sinT = np.asarray(sin, np.float32).reshape(S, HD).T
    cs2 = np.ascontiguousarray(
        np.concatenate([cosT, cosT], axis=0).astype(np.float16))
    sn2 = np.ascontiguousarray(
        np.concatenate([sinT, sinT], axis=0).astype(np.float16))

    def pack_w(w, rows):
        # [128, kt*GC + c] = w.T[kt*128 + p, c] for the row-slice of w
        wT = np.asarray(w, np.float16)[rows, :].T          # [D, GC]
        return np.ascontiguousarray(
            wT.reshape(KT, 128, GC).transpose(1, 0, 2).reshape(128, KT * GC))

    def pack_x(xbm):
        # [cb*128 + p, kt*QW + j] = x.T[kt*128 + p, cb*QW + j]
        xT = xbm.T                                         # [D, S]
        return np.ascontiguousarray(
            xT.reshape(KT, 128, QB, QW).transpose(2, 1, 0, 3)
              .reshape(QB * 128, KT * QW))

    in_maps = []
    for c in range(NCORES):
        b, g = c // GROUPS, c % GROUPS
        rows = slice(g * GC, (g + 1) * GC)
        in_maps.append({
            "xB": pack_x(x16[b]),
            "wqs": pack_w(wq, rows),
            "wks": pack_w(wk, rows),
            "wvs": pack_w(wv, rows),
            "woT": np.ascontiguousarray(
                np.asarray(wo, np.float16)[:, rows].T),
            "cs2": cs2,
            "sn2": sn2,
        })
    return in_maps


def _run(inputs, trace=False, trace_kwargs=None):
    if "nc" not in _cache:
        _cache["nc"] = _build()
    nc = _cache["nc"]
    in_maps = _shard_inputs(
        inputs["x"], inputs["cos"], inputs["sin"],
        inputs["wq"], inputs["wk"], inputs["wv"], inputs["wo"])
    res = run_bass_kernel_spmd(
        nc, in_maps, list(range(NCORES)), trace=trace,
        **(trace_kwargs or {}))
    full = np.zeros((B, S, D), dtype=np.float32)
    for c in range(NCORES):
        full[c // GROUPS] += res.results[c]["out"].astype(np.float32)
    return full, res


def kernel(**inputs):
    full, _ = _run(inputs, trace=False)
    return full


# revision 23
# speedup vs baseline: 1.1370x; 1.1370x over previous
"""Trainium2 Bass kernel for nn_Attention_84473416778449.

Reference computation (B=2, S=2048, D=1024, H=16, HD=64, fp32):
    q/k/v = x @ w{q,k,v}.T ; RoPE(q, k) ; causal softmax attention ; out @ wo.T

Sharding: 8 cores = (batch 2) x (head-group 4). Each core computes 4 heads of
one batch end-to-end and a partial output projection over its 256 channels;
the host sums the 4 partials per batch.

Final structure (evidence-driven through seven traced hardware iterations;
210.6us baseline -> 184.6us):
  - fp16 host-packed inputs and fp16 output (host converts back to f32);
    x blocks are pre-packed so each loads with a single dma descriptor
    (descriptor issue costs ~650ns of engine time each).
  - Loads spread over the three DMA-capable queues (sync/scalar/gpsimd),
    ordered by first-need so the PE never bubbles long after the ramp.
  - PE warm-up matmuls bridge the ~7us framework preamble + load phase so
    the HAM clock gate reaches 8/8 before real work and stays there.
  - Score matmuls for a head pair go to disjoint PE row groups
    (tile_position (0,0)/(64,0)) and run concurrently (measured 4ns apart);
    both write halves of one 2-bank psum tile so ONE batched exp covers the
    pair (halves the ACT per-instruction overhead). Exp runs pipelined on
    the ACT queue at ~N/1.2GHz when kept fed.
  - V tiles carry 64 replicated ones-columns per head, so the PV matmul
    emits the softmax denominator replicated across psum rows 64:128 and
    normalization is copy+reciprocal+multiply on the DVE (no gpsimd
    partition broadcast on the critical path).
  - The RoPE matmul reuses its own chain's psA tile (dead after the cos/sin
    multiplies) and is emitted a few chunks late so it never head-of-line
    blocks the PE queue; all deferred projection/output work is interleaved
    into the attention steps as fine-grained filler chunks with a rate cap
    so DVE finalize work cannot back up.
  - Negative results (measured): splitting full-K matmul chains into
    concurrent M=64 column-tiles regresses (~+26us) - LDWEIGHTS only
    overlaps in-flight matmuls across ROW groups; fp8 was rejected for
    tolerance; bf16 PSUM accumulation is TRN3-only.
"""
import sys

if "/opt/trn_rl_repo" not in sys.path:
    sys.path.insert(0, "/opt/trn_rl_repo")

import numpy as np

import concourse.bass as bass
import concourse.mybir as mybir
import concourse.tile as tile
from concourse import bacc
from concourse.bass_utils import run_bass_kernel_spmd

B, S, D, H, HD = 2, 2048, 1024, 16, 64
NCORES = 8
GROUPS = 4            # head groups
GH = H // GROUPS      # heads per group = 4
GC = GH * HD          # channels per group = 256
KT = D // 128         # 8 k-tiles over D
ST = S // 128         # 16 s-tiles
QB = 4                # sq blocks of 512
QW = S // QB          # 512
XW = KT * QW          # 4096: packed x block width

f32 = mybir.dt.float32
MMDT = mybir.dt.float16   # matmul-operand dtype
Exp = mybir.ActivationFunctionType.Exp
Copy = mybir.ActivationFunctionType.Copy

_cache = {}


def _build():
    nc = bacc.Bacc("TRN2", num_devices=NCORES)

    # host-packed: row-block cb holds [p, kt*QW + j] = x[b].T[kt*128+p, cb*QW+j]
    xB = nc.dram_tensor("xB", [QB * 128, XW], MMDT, kind="ExternalInput").ap()
    # host-packed: [p, kt*GC + c] = w.T[kt*128 + p, c]
    wqs = nc.dram_tensor("wqs", [128, KT * GC], MMDT, kind="ExternalInput").ap()
    wks = nc.dram_tensor("wks", [128, KT * GC], MMDT, kind="ExternalInput").ap()
    wvs = nc.dram_tensor("wvs", [128, KT * GC], MMDT, kind="ExternalInput").ap()
    woT = nc.dram_tensor("woT", [GC, D], MMDT, kind="ExternalInput").ap()
    cs2 = nc.dram_tensor("cs2", [128, S], MMDT, kind="ExternalInput").ap()
    sn2 = nc.dram_tensor("sn2", [128, S], MMDT, kind="ExternalInput").ap()
    out = nc.dram_tensor("out", [S, D], MMDT, kind="ExternalOutput").ap()

    with tile.TileContext(nc) as tc:
        with tc.tile_pool(name="persist", bufs=1) as pp, \
             tc.tile_pool(name="rope", bufs=3) as rp, \
             tc.tile_pool(name="probs", bufs=6) as wp, \
             tc.tile_pool(name="outsb", bufs=2) as op_, \
             tc.tile_pool(name="small", bufs=3) as sp:

            xb = [pp.tile([128, XW], MMDT, tag=f"xb{cb}", name=f"xb{cb}")
                  for cb in range(QB)]

            def xs(kt, cb):
                return xb[cb][:, kt * QW:(kt + 1) * QW]

            def load_w(src, eng):
                t = pp.tile([128, KT * GC], MMDT, tag=f"w{src.tensor.name}",
                            name=f"w{src.tensor.name}")
                eng.dma_start(t[:], src)
                return t

            # ---- loads spread across the three DMA queues by first-need ----
            warm = pp.tile([128, 128], MMDT, tag="warm")
            nc.gpsimd.memset(warm[:], 0.0)
            warm2 = pp.tile([128, QW + 128], MMDT, tag="warm2")
            nc.gpsimd.memset(warm2[:], 0.0)
            nc.gpsimd.dma_start(xb[0][:, 0:XW // 2], xB[0:128, 0:XW // 2])
            wq_s = load_w(wqs, nc.sync)
            nc.scalar.dma_start(xb[0][:, XW // 2:XW], xB[0:128, XW // 2:XW])
            cs_sb = pp.tile([128, S], MMDT, tag="cs")
            nc.sync.dma_start(cs_sb[:], cs2[:])
            sn_sb = pp.tile([128, S], MMDT, tag="sn")
            nc.scalar.dma_start(sn_sb[:], sn2[:])
            wk_s = load_w(wks, nc.gpsimd)
            wv_s = load_w(wvs, nc.scalar)
            wo_s = []
            for kt in range(2):
                t = pp.tile([128, D], MMDT, tag=f"wo{kt}", name=f"wo{kt}")
                nc.sync.dma_start(t[:], woT[kt * 128:(kt + 1) * 128, :])
                wo_s.append(t)
            nc.gpsimd.dma_start(xb[1][:], xB[128:256, :])
            nc.gpsimd.dma_start(xb[2][:], xB[256:384, :])
            nc.gpsimd.dma_start(xb[3][:], xB[384:512, :])

            # ---- PE warm-up: dummy matmuls while the first DMAs land ------
            with tc.tile_pool(name="psW", bufs=1, space="PSUM") as psW:
                wps = psW.tile([128, QW], f32, tag="wps")
                for _ in range(20):
                    nc.tensor.matmul(wps[:, 0:128], warm[:], warm[:],
                                     start=True, stop=True)
                for _ in range(26):
                    nc.tensor.matmul(wps[:], warm2[:, QW:QW + 128],
                                     warm2[:, 0:QW],
                                     start=True, stop=True)

            # ---- constants -------------------------------------------------
            cscratch = pp.tile([128, 128], f32, tag="cscratch")
            nc.gpsimd.memset(cscratch[:], 0.0)
            for blk in range(2):
                sub = cscratch[blk * 64:(blk + 1) * 64,
                               blk * 64:(blk + 1) * 64]
                nc.gpsimd.affine_select(   # -1 where p - f == 32
                    out=sub, in_=sub, pattern=[[-1, 64]], base=-32,
                    channel_multiplier=1,
                    compare_op=mybir.AluOpType.not_equal, fill=-1.0)
                nc.gpsimd.affine_select(   # +1 where f - p == 32
                    out=sub, in_=sub, pattern=[[1, 64]], base=-32,
                    channel_multiplier=-1,
                    compare_op=mybir.AluOpType.not_equal, fill=1.0)
            rt2 = pp.tile([128, 128], MMDT, tag="rt2")
            nc.vector.tensor_copy(rt2[:], cscratch[:])

            qT = [[pp.tile([128, QW], MMDT, tag=f"qT{i}_{b}",
                           name=f"qT{i}_{b}") for b in range(QB)]
                  for i in range(2)]
            kTt = [[pp.tile([128, QW], MMDT, tag=f"kT{i}_{b}",
                            name=f"kT{i}_{b}") for b in range(QB)]
                   for i in range(2)]
            attnT = [[pp.tile([128, QW], MMDT, tag=f"aT{i}_{b}",
                              name=f"aT{i}_{b}") for b in range(QB)]
                     for i in range(2)]
            # per head h: cols h*128+0:64 = v payload, h*128+64:128 = ones
            # (PV then emits the denominator replicated on psum rows 64:128)
            v_sb = [pp.tile([128, 4 * 128], MMDT, tag=f"v{i}",
                            name=f"v{i}") for i in range(ST)]
            ones256 = pp.tile([128, 256], f32, tag="ones256")
            nc.gpsimd.memset(ones256[:], 1.0)
            for vt in v_sb:
                nc.vector.tensor_copy(
                    vt[:].rearrange("p (h e) -> p h e", e=128)[:, :, HD:128],
                    ones256[:].rearrange("p (h e) -> p h e", e=HD))

            with tc.tile_pool(name="psA", bufs=2, space="PSUM") as psA, \
                 tc.tile_pool(name="psS", bufs=2, space="PSUM") as psS, \
                 tc.tile_pool(name="psO", bufs=2, space="PSUM") as psO:

                def qk_chain_chunks(w_src, dst, hp, sb):
                    """[mm x8, fin_dve] plus a separate fin_rope chunk that the
                    caller emits a few chunks later (it holds the PE rope
                    matmul, which must not wait at the head of the PE queue).
                    The rope matmul reuses the chain's own psA tile."""
                    cols = slice(sb * QW, (sb + 1) * QW)
                    box = {}

                    def mk_mm(kt):
                        def f():
                            if kt == 0:
                                box['pq'] = psA.tile(
                                    [128, QW], f32, tag="proj",
                                    name=f"pq{hp}_{sb}")
                            nc.tensor.matmul(
                                box['pq'][:],
                                w_src[:, kt * GC + hp * 128:
                                      kt * GC + hp * 128 + 128],
                                xs(kt, sb),
                                start=(kt == 0), stop=(kt == KT - 1))
                        return f

                    def fin_dve():
                        pq = box['pq']
                        box['tcs'] = rp.tile([128, QW], MMDT, tag="tcs",
                                             name=f"tcs{hp}_{sb}")
                        nc.vector.tensor_tensor(
                            out=box['tcs'][:], in0=pq[:], in1=cs_sb[:, cols],
                            op=mybir.AluOpType.mult)
                        box['tsn'] = rp.tile([128, QW], MMDT, tag="tsn",
                                             name=f"tsn{hp}_{sb}")
                        nc.vector.tensor_tensor(
                            out=box['tsn'][:], in0=pq[:], in1=sn_sb[:, cols],
                            op=mybir.AluOpType.mult)

                    def fin_rope():
                        pq = box['pq']
                        nc.tensor.matmul(pq[:], rt2[:], box['tsn'][:],
                                         start=True, stop=True)
                        nc.vector.tensor_tensor(
                            out=dst[hp][sb][:], in0=pq[:], in1=box['tcs'][:],
                            op=mybir.AluOpType.add)
                    return [mk_mm(kt) for kt in range(KT)] + [fin_dve], fin_rope

                def v_chain_chunks(st):
                    box = {}

                    def mk_mm(kt):
                        def f():
                            if kt == 0:
                                box['pv'] = psA.tile(
                                    [128, QW], f32, tag="proj",
                                    name=f"pv_{st}")
                            nc.tensor.matmul(
                                box['pv'][:, 0:GC],
                                xs(kt, st // 4)[:, (st % 4) * 128:
                                                (st % 4) * 128 + 128],
                                wv_s[:, kt * GC:(kt + 1) * GC],
                                start=(kt == 0), stop=(kt == KT - 1))
                        return f

                    def fin():
                        vhe = v_sb[st][:].rearrange("p (h e) -> p h e", e=128)
                        nc.scalar.activation(
                            vhe[:, :, 0:HD],
                            box['pv'][:, 0:GC].rearrange(
                                "p (h d) -> p h d", d=HD),
                            Copy)
                    return [mk_mm(kt) for kt in range(KT)] + [fin]

                def proj_chunks(sbx):
                    """Chunk list for block sbx with each chain's rope matmul
                    deferred into the following chain's matmul stream."""
                    chunks = []
                    pending = []

                    def flush():
                        while pending:
                            chunks.append(pending.pop(0))

                    for hp in range(2):
                        for w_src, dst in ((wq_s, qT), (wk_s, kTt)):
                            main, fin_rope = qk_chain_chunks(
                                w_src, dst, hp, sbx)
                            chunks += main[:3]
                            flush()
                            chunks += main[3:]
                            pending.append(fin_rope)
                    for st in range(sbx * 4, sbx * 4 + 4):
                        vc = v_chain_chunks(st)
                        chunks += vc[:3]
                        flush()
                        chunks += vc[3:]
                    return chunks

                def out_chunks(st, on_dve=False):
                    box = {}

                    def mms(db):
                        def f():
                            box[db] = psA.tile([128, QW], f32, tag="proj",
                                               name=f"pc_{st}_{db}")
                            if db == 0:
                                box['ob'] = op_.tile([128, 2 * QW], MMDT,
                                                     tag="outsb",
                                                     name=f"ob_{st}")
                            for kt in range(2):
                                nc.tensor.matmul(
                                    box[db][:],
                                    attnT[kt][st // 4][:, (st % 4) * 128:
                                                       (st % 4) * 128 + 128],
                                    wo_s[kt][:, db * QW:(db + 1) * QW],
                                    start=(kt == 0), stop=(kt == 1))
                        return f

                    def cp(db):
                        def f():
                            if on_dve:
                                nc.vector.tensor_copy(
                                    box['ob'][:, db * QW:(db + 1) * QW],
                                    box[db][:])
                            else:
                                nc.scalar.activation(
                                    box['ob'][:, db * QW:(db + 1) * QW],
                                    box[db][:], Copy)
                            if db == 1:
                                nc.sync.dma_start(
                                    out[st * 128:(st + 1) * 128, :],
                                    box['ob'][:])
                        return f
                    return [mms(0), cp(0), mms(1), cp(1)]

                def merge_prop(a, b):
                    res, ia, ib = [], 0, 0
                    la, lb = max(len(a), 1), max(len(b), 1)
                    while ia < len(a) or ib < len(b):
                        if ib >= len(b) or (ia < len(a) and
                                            ia * lb <= ib * la):
                            res.append(a[ia]); ia += 1
                        else:
                            res.append(b[ib]); ib += 1
                    return res

                def emit_attention(qb, fillers):
                    nsk = (qb + 1) * 4
                    steps_total = 2 * nsk
                    step = 0
                    fi = 0
                    for hp in range(2):
                        hA, hB = 2 * hp, 2 * hp + 1
                        poA = psO.tile([128, QW], f32, tag="pvacc",
                                       name=f"poA_{hp}_{qb}")
                        poB = psO.tile([128, QW], f32, tag="pvacc",
                                       name=f"poB_{hp}_{qb}")

                        def pv_pair(kt, c0, cw, prt):
                            nc.tensor.matmul(
                                poA[:, c0:QW],
                                v_sb[kt][:, hA * 128:hA * 128 + 128],
                                prt[:, 0:cw],
                                start=(kt == 0), stop=(kt == nsk - 1))
                            nc.tensor.matmul(
                                poB[:, c0:QW],
                                v_sb[kt][:, hB * 128:hB * 128 + 128],
                                prt[:, QW:QW + cw],
                                start=(kt == 0), stop=(kt == nsk - 1))

                        for kt in range(nsk):
                            c0 = max(0, kt * 128 - qb * QW)
                            cw = QW - c0
                            ps2 = psS.tile([128, 2 * QW], f32, tag="score",
                                           name=f"sc_{hp}_{qb}_{kt}")
                            nc.tensor.matmul(
                                ps2[:, 0:cw],
                                kTt[hp][kt // 4][0:64,
                                                 (kt % 4) * 128:
                                                 (kt % 4) * 128 + 128],
                                qT[hp][qb][0:64, c0:QW],
                                start=True, stop=True,
                                tile_position=(0, 0))
                            nc.tensor.matmul(
                                ps2[:, QW:QW + cw],
                                kTt[hp][kt // 4][64:128,
                                                 (kt % 4) * 128:
                                                 (kt % 4) * 128 + 128],
                                qT[hp][qb][64:128, c0:QW],
                                start=True, stop=True,
                                tile_position=(64, 0))
                            prt = wp.tile([128, 2 * QW], MMDT, tag="probs",
                                          name=f"pr_{hp}_{qb}_{kt}")
                            nc.scalar.activation(
                                prt[:].rearrange(
                                    "p (b j) -> p b j", b=2)[:, :, 0:cw],
                                ps2[:].rearrange(
                                    "p (b j) -> p b j", b=2)[:, :, 0:cw],
                                Exp, scale=0.125)
                            if kt >= nsk - 4:
                                for pv in (prt[:, 0:cw],
                                           prt[:, QW:QW + cw]):
                                    nc.gpsimd.affine_select(
                                        out=pv, in_=pv,
                                        pattern=[[1, cw]], base=0,
                                        channel_multiplier=-1,
                                        compare_op=mybir.AluOpType.is_ge,
                                        fill=0.0)
                            # one filler matmul covers the exp latency before
                            # this step's PV; the rest follow the PV
                            step += 1
                            want = (len(fillers) * step) // steps_total
                            if fi < want:
                                fillers[fi]()
                                fi += 1
                            pv_pair(kt, c0, cw, prt)
                            while fi < want:
                                fillers[fi]()
                                fi += 1
                        for h, po in ((hA, poA), (hB, poB)):
                            den_s = sp.tile([64, QW], f32, tag="dens")
                            nc.vector.tensor_copy(den_s[:], po[64:128, :])
                            rcb = sp.tile([64, QW], f32, tag="recb")
                            nc.vector.reciprocal_approx_fast(
                                out=rcb[:], in_=den_s[:])
                            nc.vector.tensor_tensor(
                                out=attnT[hp][qb][(h % 2) * 64:
                                                  (h % 2) * 64 + 64, :],
                                in0=po[0:HD, :], in1=rcb[:],
                                op=mybir.AluOpType.mult)
                    while fi < len(fillers):
                        fillers[fi]()
                        fi += 1

                # sb=0 projections are a dense block (nothing to hide behind)
                for ch in proj_chunks(0):
                    ch()
                for sb in range(QB):
                    steps = 2 * (sb + 1) * 4
                    outs = []
                    if sb > 0:
                        for st in range((sb - 1) * 4, sb * 4):
                            outs += out_chunks(st)
                    rest = []
                    projs = []
                    if sb + 1 < QB:
                        pch = proj_chunks(sb + 1)
                        # cap interleave rate so deferred DVE work cannot
                        # back up behind the PE racing through fillers
                        ncap = max(0, 2 * steps - len(outs))
                        projs = pch[:ncap]
                        rest = pch[ncap:]
                    emit_attention(sb, merge_prop(projs, outs))
                    for ch in rest:
                        ch()
                for st in range((QB - 1) * 4, QB * 4):
                    for ch in out_chunks(st, on_dve=True):
                        ch()

    nc.compile()
    return nc


def _shard_inputs(x, cos, sin, wq, wk, wv, wo):
    x16 = np.asarray(x, dtype=np.float16)
    cosT = np.asarray(cos, np.float32).reshape(S, HD).T
    sinT = np.asarray(sin, np.float32).reshape(S, HD).T
    cs2 = np.ascontiguousarray(
        np.concatenate([cosT, cosT], axis=0).astype(np.float16))
    sn2 = np.ascontiguousarray(
        np.concatenate([sinT, sinT], axis=0).astype(np.float16))

    def pack_w(w, rows):
        # [128, kt*GC + c] = w.T[kt*128 + p, c] for the row-slice of w
        wT = np.asarray(w, np.float16)[rows, :].T          # [D, GC]
        return np.ascontiguousarray(
            wT.reshape(KT, 128, GC).transpose(1, 0, 2).reshape(128, KT * GC))

    def pack_x(xbm):
        # [cb*128 + p, kt*QW + j] = x.T[kt*128 + p, cb*QW + j]
        xT = xbm.T                                         # [D, S]
        return np.ascontiguousarray(
            xT.reshape(KT, 128, QB, QW).transpose(2, 1, 0, 3)
              .reshape(QB * 128, KT * QW))

    in_maps = []
    for c in range(NCORES):
        b, g = c // GROUPS, c % GROUPS
        rows = slice(g * GC, (g + 1) * GC)
        in_maps.append({
            "xB": pack_x(x16[b]),
            "wqs": pack_w(wq, rows),
            "wks": pack_w(wk, rows),
            "wvs": pack_w(wv, rows),
            "woT": np.ascontiguousarray(
                np.asarray(wo, np.float16)[:, rows].T),
            "cs2": cs2,
            "sn2": sn2,
        })
    return in_maps


def _run(inputs, trace=False, trace_kwargs=None):
    if "nc" not in _cache:
        _cache["nc"] = _build()
    nc = _cache["nc"]
    in_maps = _shard_inputs(
        inputs["x"], inputs["cos"], inputs["sin"],
        inputs["wq"], inputs["wk"], inputs["wv"], inputs["wo"])
    res = run_bass_kernel_spmd(
        nc, in_maps, list(range(NCORES)), trace=trace,
        **(trace_kwargs or {}))
    full = np.zeros((B, S, D), dtype=np.float32)
    for c in range(NCORES):
        full[c // GROUPS] += res.results[c]["out"].astype(np.float32)
    return full, res


def kernel(**inputs):
    full, _ = _run(inputs, trace=False)
    return full
